# revision 52
# baseline (speedup 1.0000x reference)
"""Causal self-attention Trainium2 kernel — tensor-parallel over heads on 8 NeuronCores.

Problem: B=4, T=2048, C=1024, H=16 heads (head_dim 64), fp32 in/out.
Sharding: 2 heads per core. Each core computes the qkv projection for its
128 head-dim columns, full causal attention for its 2 heads, and a partial
output projection (its 128 W_proj rows); fp16 partials are summed on host.

All matmul operands are fp16 (PE runs fp16 at 1 cycle/row, same as bf16,
with 10-bit mantissa); accumulation stays fp32 in PSUM, softmax
normalization in fp32. Engine assignment is explicit: scalar = exp + q/k
bias evacuation, vector = copies/normalize, gpsimd = causal mask +
partition broadcast + x loads, sync = output stores.
"""

import numpy as np

import concourse.bass as bass
import concourse.mybir as mybir
from concourse import bacc
from concourse.tile import TileContext
from concourse.masks import make_identity

# NOTE: walrus's --enable-ldw-opt rejects fp16 LDWEIGHTS ("InstLdweights is
# not compatible with LDW optimization"), so unlike the fp32r variant we run
# with the default ldw-opt=false.

F32 = mybir.dt.float32
F16 = mybir.dt.float16
F8 = mybir.dt.float8e4
AF = mybir.ActivationFunctionType

# fp8 DoubleRow QKV projection: x and W_attn are fed to the PE as fp8(e4m3)
# in DoubleRow pair-interleaved layout (2x matmul throughput).  W is
# prescaled by WS on the host to sit in fp8 range; the factor is folded out
# via the exp scale (q.k picks up WS^2) and W_proj (y picks up WS).
# Measured on the real inputs: fp8 q/k loses ~4.5e-2 rel err (the exp
# amplifies operand rounding), so this stays off.
FP8_QKV = False
WS = 32.0

B, T, C, H = 4, 2048, 1024, 16
HD = 64
NCORES = 8
CT = C // 128          # 8 contraction tiles
QT = 512               # q tile (free dim of S^T matmuls)
KT = 128               # k tile (partition dim of S^T)
SCALE = 1.0 / np.sqrt(HD)
# v subtile layout: [vA(64) | onesA(1) | onesB(1) | junk(63) | vB(64)].
# Head A's PV stationary is cols 0:65 ([vA|onesA] -> ya_A dims at psum
# partitions 0-63, sum at 64). Head B's is cols 65:193
# ([onesB|junk63|vB] -> sum at partition 0, dims at partitions 64-127), so
# the normalized y of head B can be written to yT[64:128] without any
# cross-partition move.
VW = 193

_CACHED = {}


def build_kernel(b=B, t=T):
    """Build the per-core SPMD program. t must be a multiple of 512."""
    from concourse import hw_specs

    # Calibrate the tile scheduler's cost model to measured per-op rates
    # (NTFF traces show ACT ~13% and Pool ~2x above the stock model); this
    # only changes the static instruction interleave, not semantics.
    # Restored after compile so nothing leaks.
    _ET = mybir.EngineType
    _saved = dict(hw_specs.TRN2Spec.CYCLE_T)
    hw_specs.TRN2Spec.CYCLE_T[_ET.Activation] = 1e9 / 0.95e9
    hw_specs.TRN2Spec.CYCLE_T[_ET.Pool] = 1e9 / 0.6e9
    try:
        return _build_kernel_inner(b, t)
    finally:
        hw_specs.TRN2Spec.CYCLE_T.clear()
        hw_specs.TRN2Spec.CYCLE_T.update(_saved)


def _build_kernel_inner(b, t):
    assert t % QT == 0
    nq = t // QT           # q-tiles per sequence
    nst = t // 128         # 128-token subtiles per sequence
    bt = b * t

    nc = bacc.Bacc("TRN2", target_bir_lowering=False, debug=False,
                   num_devices=NCORES)

    if FP8_QKV:
        # x in DoubleRow layout: [ct*64+p, j, tok] = x[tok, ct*128+2p+j]
        xT = nc.dram_tensor("xT", [C // 2, 2, bt], F8, kind="ExternalInput")
        # W in DoubleRow lhsT layout: [p, ct, j, m] = WS*W[ct*128+2p+j, m]
        wq = nc.dram_tensor("wq", [64, 2 * C], F8, kind="ExternalInput")
        wk = nc.dram_tensor("wk", [64, 2 * C], F8, kind="ExternalInput")
        wv = nc.dram_tensor("wv", [64, 2 * C], F8, kind="ExternalInput")
    else:
        xT = nc.dram_tensor("xT", [C, bt], F16, kind="ExternalInput")
        # weights in lhsT layout ([p, ct*128+m] = W[ct*128+p, m]),
        # pre-arranged on host so the load is one contiguous DMA
        wq = nc.dram_tensor("wq", [128, C], F16, kind="ExternalInput")
        wk = nc.dram_tensor("wk", [128, C], F16, kind="ExternalInput")
        wv = nc.dram_tensor("wv", [128, C], F16, kind="ExternalInput")
    wp = nc.dram_tensor("wp", [128, C], F16, kind="ExternalInput")
    bq = nc.dram_tensor("bq", [128, 1], F32, kind="ExternalInput")
    bk = nc.dram_tensor("bk", [128, 1], F32, kind="ExternalInput")
    out = nc.dram_tensor("out", [bt, C], F16, kind="ExternalOutput")

    with TileContext(nc) as tc:
        with (
            tc.tile_pool(name="const", bufs=1) as constp,
            tc.tile_pool(name="xin", bufs=4) as xin,
            tc.tile_pool(name="qk", bufs=3) as qkp,
            tc.tile_pool(name="es", bufs=4) as esp,
            tc.tile_pool(name="yt", bufs=2) as ytp,
            tc.tile_pool(name="small", bufs=3) as smallp,
            tc.tile_pool(name="outsb", bufs=3) as outp,
            tc.tile_pool(name="ps_s", bufs=2, space="PSUM") as ps_s,
            tc.tile_pool(name="ps_ya", bufs=2, space="PSUM") as ps_ya,
            tc.tile_pool(name="ps_mm", bufs=2, space="PSUM") as ps_mm,
        ):
            # ---- constants / weights ----
            ident = constp.tile([128, 128], F16, tag="ident")
            make_identity(nc, ident[:])
            if FP8_QKV:
                wq_sb = constp.tile([64, CT, 2, 128], F8, tag="wq")
                wk_sb = constp.tile([64, CT, 2, 128], F8, tag="wk")
                wv_sb = constp.tile([64, CT, 2, 128], F8, tag="wv")
                for w_dram, w_sb in ((wk, wk_sb), (wq, wq_sb), (wv, wv_sb)):
                    nc.gpsimd.dma_start(
                        out=w_sb[:],
                        in_=w_dram[:].rearrange(
                            "p (ct j m) -> p ct j m", ct=CT, j=2))
            else:
                wq_sb = constp.tile([128, C], F16, tag="wq")
                wk_sb = constp.tile([128, C], F16, tag="wk")
                wv_sb = constp.tile([128, C], F16, tag="wv")
                # k first (first consumer), on the fast sync HWDGE ring
                nc.sync.dma_start(out=wk_sb[:], in_=wk[:])
                nc.gpsimd.dma_start(out=wq_sb[:], in_=wq[:])
                nc.gpsimd.dma_start(out=wv_sb[:], in_=wv[:])
            wp_sb = constp.tile([128, C], F16, tag="wp")
            nc.gpsimd.dma_start(out=wp_sb[:], in_=wp[:])
            bq_sb = constp.tile([128, 1], F32, tag="bq")
            bk_sb = constp.tile([128, 1], F32, tag="bk")
            nc.gpsimd.dma_start(out=bq_sb[:], in_=bq[:])
            nc.gpsimd.dma_start(out=bk_sb[:], in_=bk[:])
            def emit_proj(pbi, yT_tile, pqt):
                # output projection for q-tile pqt (2 subtiles per DMA);
                # PSUM evacuation split between scalar and vector engines
                for sp in range(QT // 256):
                    osb = outp.tile([128, 2, C], F16, tag="osb")
                    for sj in range(2):
                        st = pqt * (QT // 128) + sp * 2 + sj
                        for n in range(C // QT):
                            pp = ps_mm.tile([128, QT], F32, tag="mm")
                            nc.tensor.matmul(
                                pp[:],
                                yT_tile[:, st * 128:(st + 1) * 128],
                                wp_sb[:, n * QT:(n + 1) * QT],
                                start=True, stop=True)
                            if n == 1 and sj == 1:
                                nc.scalar.activation(
                                    osb[:, sj, n * QT:(n + 1) * QT],
                                    pp[:], AF.Copy)
                            else:
                                nc.vector.tensor_copy(
                                    out=osb[:, sj, n * QT:(n + 1) * QT],
                                    in_=pp[:])
                    r0 = pbi * t + (pqt * (QT // 128) + sp * 2) * 128
                    nc.sync.dma_start(
                        out=out[r0:r0 + 256, :].rearrange(
                            "(s p) c -> p s c", p=128),
                        in_=osb[:])

            def emit_qkv(bi):
                # v_sb per 128-token subtile (VW layout, see above)
                v_sb = qkp.tile([128, nst * VW], F16, tag="v")
                v_view = v_sb[:].rearrange("p (s w) -> p s w", w=VW)
                nc.vector.memset(v_view[:, :, 64:66], 1.0)
                nc.vector.memset(v_view[:, :, 66:129], 0.0)
                qT_sb = qkp.tile([128, t], F16, tag="qT")
                kT_sb = qkp.tile([128, t], F16, tag="kT")

                xts = []
                for colt in range(t // QT):
                    if FP8_QKV:
                        xt = xin.tile([64, CT, 2, QT], F8, tag="xt")
                    else:
                        xt = xin.tile([128, CT, QT], F16, tag="xt")
                    csl = slice(bi * t + colt * QT, bi * t + (colt + 1) * QT)
                    if FP8_QKV:
                        if bi == 0 and colt == 0:
                            for ct in range(CT):
                                nc.sync.dma_start(
                                    out=xt[:, ct],
                                    in_=xT[ct * 64:(ct + 1) * 64, :, csl])
                        else:
                            nc.sync.dma_start(
                                out=xt[:],
                                in_=xT[:, :, csl].rearrange(
                                    "(ct p) j n -> p ct j n", p=64))
                    elif bi == 0 and colt == 0:
                        # split the very first load per contraction tile so
                        # the first matmul can start as soon as ct=0 lands
                        for ct in range(CT):
                            nc.sync.dma_start(
                                out=xt[:, ct, :],
                                in_=xT[ct * 128:(ct + 1) * 128, csl])
                    else:
                        nc.sync.dma_start(
                            out=xt[:],
                            in_=xT[:, csl].rearrange(
                                "(ct p) n -> p ct n", p=128))
                    xts.append(xt)

                def qkv_mm(ps, w_sb, xt, ct):
                    if FP8_QKV:
                        nc.tensor.matmul(
                            ps[:], w_sb[:, ct], xt[:, ct],
                            start=(ct == 0), stop=(ct == CT - 1),
                            perf_mode=mybir.MatmulPerfMode.DoubleRow)
                    else:
                        nc.tensor.matmul(
                            ps[:], w_sb[:, ct * 128:(ct + 1) * 128],
                            xt[:, ct, :],
                            start=(ct == 0), stop=(ct == CT - 1))

                for colt in range(t // QT):
                    xt = xts[colt]
                    csl = slice(colt * QT, (colt + 1) * QT)
                    for w_sb, dst, bias in (
                        (wk_sb, kT_sb, bk_sb), (wq_sb, qT_sb, bq_sb),
                    ):
                        ps = ps_mm.tile([128, QT], F32, tag="mm")
                        for ct in range(CT):
                            qkv_mm(ps, w_sb, xt, ct)
                        nc.vector.tensor_scalar_add(
                            out=dst[:, csl], in0=ps[:], scalar1=bias[:])
                    # V^T for this col tile, then transpose to natural layout
                    ps = ps_mm.tile([128, QT], F32, tag="mm")
                    for ct in range(CT):
                        qkv_mm(ps, wv_sb, xt, ct)
                    vt_col = smallp.tile([128, QT], F16, tag="vtcol")
                    nc.vector.tensor_copy(out=vt_col[:], in_=ps[:])
                    for sj in range(QT // 128):
                        st = colt * (QT // 128) + sj
                        vt_ps = ps_mm.tile([128, 128], F16, tag="mm")
                        nc.tensor.transpose(
                            vt_ps[:], vt_col[:, sj * 128:(sj + 1) * 128],
                            ident[:])
                        nc.vector.tensor_copy(
                            out=v_view[:, st, 0:64], in_=vt_ps[:, 0:64])
                        nc.vector.tensor_copy(
                            out=v_view[:, st, 129:193], in_=vt_ps[:, 64:128])
                return qT_sb, kT_sb, v_view

            def emit_attention(bi, qT_sb, kT_sb, v_view):
                nonlocal pending
                yT_sb = ytp.tile([128, t], F16, tag="yT")
                for qt in range(nq):
                    n_k = (qt + 1) * (QT // KT)   # k-tiles of 128
                    q0 = qt * QT
                    ya_a = ps_ya.tile([65, QT], F32, tag="ya", name="yaA")
                    ya_b = ps_ya.tile([128, QT], F32, tag="ya", name="yaB")
                    for kt in range(n_k):
                        lo = max(0, kt * KT - q0)
                        # S^T for both heads: head A on PE rows 0-63,
                        # head B on rows 64-127.
                        sg = ps_s.tile([128, 2 * QT], F32, tag="sg")
                        es = esp.tile([128, 2 * QT], F16, tag="es")
                        for h in range(2):
                            hsl = slice(h * 64, (h + 1) * 64)
                            nc.tensor.matmul(
                                sg[:, h * QT + lo:(h + 1) * QT],
                                kT_sb[hsl, kt * KT:(kt + 1) * KT],
                                qT_sb[hsl, q0 + lo:q0 + QT],
                                start=True, stop=True,
                            )
                        sg_v = sg[:].rearrange("p (h q) -> p h q", h=2)
                        es_v = es[:].rearrange("p (h q) -> p h q", h=2)
                        nc.scalar.activation(
                            es_v[:, :, lo:], sg_v[:, :, lo:],
                            AF.Exp,
                            scale=SCALE / (WS * WS) if FP8_QKV else SCALE)
                        if kt * KT >= q0:
                            # causal band select, both heads in one op
                            nc.gpsimd.affine_select(
                                out=es_v[:, :, lo:lo + KT],
                                in_=es_v[:, :, lo:lo + KT],
                                compare_op=mybir.AluOpType.is_ge,
                                fill=0.0,
                                base=0,
                                channel_multiplier=-1,
                                pattern=[[0, 2], [1, KT]],
                            )
                        nc.tensor.matmul(
                            ya_a[:, lo:QT],
                            v_view[:, kt, 0:65],
                            es[:, lo:QT],
                            start=(kt == 0), stop=(kt == n_k - 1),
                        )
                        nc.tensor.matmul(
                            ya_b[:, lo:QT],
                            v_view[:, kt, 65:193],
                            es[:, QT + lo:2 * QT],
                            start=(kt == 0), stop=(kt == n_k - 1),
                        )
                    if len(pending) >= 2:
                        emit_proj(*pending.pop(0))
                    # normalize: y = ya / sum.  Head A's sum sits at psum
                    # partition 64 (scalar engine hops it to partition 0);
                    # head B's sum is already at partition 0.
                    srow = smallp.tile([1, QT], F32, tag="srow")
                    nc.scalar.activation(
                        srow[0:1, :], ya_a[64:65, :], AF.Copy)
                    # head B's sum is at partition 0 -> plain vector copy
                    srow_b = smallp.tile([1, QT], F32, tag="srowb")
                    nc.vector.tensor_copy(
                        out=srow_b[0:1, :], in_=ya_b[0:1, :])
                    # [1/sumA | 1/sumB] in one row, one broadcast to all 128
                    # partitions (gpsimd only runs affine_select besides this,
                    # so its queue stays short)
                    rr = smallp.tile([1, 2 * QT], F32, tag="rr")
                    nc.vector.reciprocal_approx_fast(
                        out=rr[0:1, QT:2 * QT], in_=srow_b[0:1, :])
                    nc.vector.reciprocal_approx_fast(
                        out=rr[0:1, 0:QT], in_=srow[0:1, :])
                    bc = smallp.tile([128, 2 * QT], F32, tag="bc")
                    nc.gpsimd.partition_broadcast(
                        bc[:], rr[0:1, :], channels=128)
                    nc.vector.tensor_mul(
                        out=yT_sb[0:64, q0:q0 + QT],
                        in0=ya_a[0:64, :], in1=bc[0:64, 0:QT])
                    nc.vector.tensor_mul(
                        out=yT_sb[64:128, q0:q0 + QT],
                        in0=ya_b[64:128, :], in1=bc[64:128, QT:2 * QT])
                    pending.append((bi, yT_sb, qt))

            # Software-pipeline one full batch ahead: qkv(bi+1) is emitted
            # before attention(bi), so the attention phase always has a deep
            # well of ready tensor work and never couples to fresh qkv.
            pending = []   # (bi, yT_tile, qt) whose proj is not yet emitted
            tiles = [None] * b
            tiles[0] = emit_qkv(0)
            for bi in range(b):
                if bi + 1 < b:
                    tiles[bi + 1] = emit_qkv(bi + 1)
                emit_attention(bi, *tiles[bi])
                tiles[bi] = None
            for p in pending:
                emit_proj(*p)

    nc.compile()
    return nc


NPF8 = mybir.dt.np(F8)


def _lhsT(w):
    """[C, 128] weight slice -> lhsT layout [128, C]."""
    return np.ascontiguousarray(
        w.reshape(CT, 128, 128).transpose(1, 0, 2).reshape(128, C)
    ).astype(np.float16)


def _lhsT8(w):
    """[C, 128] weight slice -> DoubleRow lhsT layout [64, 2C] fp8."""
    return np.ascontiguousarray(
        (w * WS).reshape(CT, 64, 2, 128).transpose(1, 0, 2, 3)
        .reshape(64, 2 * C)).astype(NPF8)


def _prep_inputs(x, W_attn, b_attn, W_proj, b_proj, b, t):
    bt = b * t
    if FP8_QKV:
        xT_full = np.ascontiguousarray(
            x.reshape(bt, CT, 64, 2).transpose(1, 2, 3, 0)
            .reshape(C // 2, 2, bt)).astype(NPF8)
        bscale = WS
        wp_full = (W_proj / WS).astype(np.float16)
        mk_w = _lhsT8
    else:
        xT_full = np.ascontiguousarray(
            x.reshape(bt, C).T).astype(np.float16)
        bscale = 1.0
        wp_full = W_proj.astype(np.float16)
        mk_w = _lhsT
    in_maps = []
    for c in range(NCORES):
        sl = slice(c * 128, (c + 1) * 128)
        in_maps.append({
            "xT": xT_full,
            "wq": mk_w(W_attn[:, sl]),
            "wk": mk_w(W_attn[:, 1024:2048][:, sl]),
            "wv": mk_w(W_attn[:, 2048:3072][:, sl]),
            "wp": np.ascontiguousarray(wp_full[sl, :]),
            "bq": np.ascontiguousarray(
                b_attn[sl].reshape(128, 1) * bscale).astype(np.float32),
            "bk": np.ascontiguousarray(
                b_attn[1024:2048][sl].reshape(128, 1) * bscale
            ).astype(np.float32),
        })
    return in_maps


def kernel(x, W_attn, b_attn, W_proj, b_proj, _trace=False):
    from concourse.bass_utils import run_bass_kernel_spmd

    x = np.asarray(x, dtype=np.float32)
    W_attn = np.asarray(W_attn, dtype=np.float32)
    b_attn = np.asarray(b_attn, dtype=np.float32)
    W_proj = np.asarray(W_proj, dtype=np.float32)
    b_proj = np.asarray(b_proj, dtype=np.float32)
    b, t, c = x.shape

    key = (b, t)
    if key not in _CACHED:
        _CACHED[key] = build_kernel(b, t)
    nc = _CACHED[key]

    in_maps = _prep_inputs(x, W_attn, b_attn, W_proj, b_proj, b, t)
    res = run_bass_kernel_spmd(
        nc, in_maps, core_ids=list(range(NCORES)), trace=_trace)

    acc = res.results[0]["out"].astype(np.float32)
    for r in res.results[1:]:
        acc = acc + r["out"].astype(np.float32)
    acc += b_attn[2048:3072] @ W_proj + b_proj
    out = acc.reshape(b, t, c)
    if _trace:
        kernel.last_result = res
    return out


# revision 55
# speedup vs baseline: 1.0019x; 1.0019x over previous
"""Causal self-attention Trainium2 kernel — tensor-parallel over heads on 8 NeuronCores.

Problem: B=4, T=2048, C=1024, H=16 heads (head_dim 64), fp32 in/out.
Sharding: 2 heads per core. Each core computes the qkv projection for its
128 head-dim columns, full causal attention for its 2 heads, and a partial
output projection (its 128 W_proj rows); fp16 partials are summed on host.

All matmul operands are fp16 (PE runs fp16 at 1 cycle/row, same as bf16,
with 10-bit mantissa); accumulation stays fp32 in PSUM, softmax
normalization in fp32. Engine assignment is explicit: scalar = exp + q/k
bias evacuation, vector = copies/normalize, gpsimd = causal mask +
partition broadcast + x loads, sync = output stores.
"""

import numpy as np

import concourse.bass as bass
import concourse.mybir as mybir
from concourse import bacc
from concourse.tile import TileContext
from concourse.masks import make_identity

# NOTE: walrus's --enable-ldw-opt rejects fp16 LDWEIGHTS ("InstLdweights is
# not compatible with LDW optimization"), so unlike the fp32r variant we run
# with the default ldw-opt=false.

F32 = mybir.dt.float32
F16 = mybir.dt.float16
F8 = mybir.dt.float8e4
AF = mybir.ActivationFunctionType

# fp8 DoubleRow QKV projection: x and W_attn are fed to the PE as fp8(e4m3)
# in DoubleRow pair-interleaved layout (2x matmul throughput).  W is
# prescaled by WS on the host to sit in fp8 range; the factor is folded out
# via the exp scale (q.k picks up WS^2) and W_proj (y picks up WS).
# Measured on the real inputs: fp8 q/k loses ~4.5e-2 rel err (the exp
# amplifies operand rounding), so this stays off.
FP8_QKV = False
WS = 32.0

B, T, C, H = 4, 2048, 1024, 16
HD = 64
NCORES = 8
CT = C // 128          # 8 contraction tiles
QT = 512               # q tile (free dim of S^T matmuls)
KT = 128               # k tile (partition dim of S^T)
SCALE = 1.0 / np.sqrt(HD)
# v subtile layout: [vA(64) | onesA(1) | onesB(1) | junk(63) | vB(64)].
# Head A's PV stationary is cols 0:65 ([vA|onesA] -> ya_A dims at psum
# partitions 0-63, sum at 64). Head B's is cols 65:193
# ([onesB|junk63|vB] -> sum at partition 0, dims at partitions 64-127), so
# the normalized y of head B can be written to yT[64:128] without any
# cross-partition move.
VW = 193

_CACHED = {}


def build_kernel(b=B, t=T):
    """Build the per-core SPMD program. t must be a multiple of 512."""
    from concourse import hw_specs

    # Calibrate the tile scheduler's cost model to measured per-op rates
    # (NTFF traces show ACT ~13% and Pool ~2x above the stock model); this
    # only changes the static instruction interleave, not semantics.
    # Restored after compile so nothing leaks.
    _ET = mybir.EngineType
    _saved = dict(hw_specs.TRN2Spec.CYCLE_T)
    hw_specs.TRN2Spec.CYCLE_T[_ET.Activation] = 1e9 / 1.05e9
    hw_specs.TRN2Spec.CYCLE_T[_ET.Pool] = 1e9 / 0.6e9
    try:
        return _build_kernel_inner(b, t)
    finally:
        hw_specs.TRN2Spec.CYCLE_T.clear()
        hw_specs.TRN2Spec.CYCLE_T.update(_saved)


def _build_kernel_inner(b, t):
    assert t % QT == 0
    nq = t // QT           # q-tiles per sequence
    nst = t // 128         # 128-token subtiles per sequence
    bt = b * t

    nc = bacc.Bacc("TRN2", target_bir_lowering=False, debug=False,
                   num_devices=NCORES)

    if FP8_QKV:
        # x in DoubleRow layout: [ct*64+p, j, tok] = x[tok, ct*128+2p+j]
        xT = nc.dram_tensor("xT", [C // 2, 2, bt], F8, kind="ExternalInput")
        # W in DoubleRow lhsT layout: [p, ct, j, m] = WS*W[ct*128+2p+j, m]
        wq = nc.dram_tensor("wq", [64, 2 * C], F8, kind="ExternalInput")
        wk = nc.dram_tensor("wk", [64, 2 * C], F8, kind="ExternalInput")
        wv = nc.dram_tensor("wv", [64, 2 * C], F8, kind="ExternalInput")
    else:
        xT = nc.dram_tensor("xT", [C, bt], F16, kind="ExternalInput")
        # weights in lhsT layout ([p, ct*128+m] = W[ct*128+p, m]),
        # pre-arranged on host so the load is one contiguous DMA
        wq = nc.dram_tensor("wq", [128, C], F16, kind="ExternalInput")
        wk = nc.dram_tensor("wk", [128, C], F16, kind="ExternalInput")
        wv = nc.dram_tensor("wv", [128, C], F16, kind="ExternalInput")
    wp = nc.dram_tensor("wp", [128, C], F16, kind="ExternalInput")
    bq = nc.dram_tensor("bq", [128, 1], F32, kind="ExternalInput")
    bk = nc.dram_tensor("bk", [128, 1], F32, kind="ExternalInput")
    out = nc.dram_tensor("out", [bt, C], F16, kind="ExternalOutput")

    with TileContext(nc) as tc:
        with (
            tc.tile_pool(name="const", bufs=1) as constp,
            tc.tile_pool(name="xin", bufs=4) as xin,
            tc.tile_pool(name="qk", bufs=3) as qkp,
            tc.tile_pool(name="es", bufs=4) as esp,
            tc.tile_pool(name="yt", bufs=2) as ytp,
            tc.tile_pool(name="small", bufs=3) as smallp,
            tc.tile_pool(name="outsb", bufs=3) as outp,
            tc.tile_pool(name="ps_s", bufs=2, space="PSUM") as ps_s,
            tc.tile_pool(name="ps_ya", bufs=2, space="PSUM") as ps_ya,
            tc.tile_pool(name="ps_mm", bufs=2, space="PSUM") as ps_mm,
        ):
            # ---- constants / weights ----
            ident = constp.tile([128, 128], F16, tag="ident")
            make_identity(nc, ident[:])
            if FP8_QKV:
                wq_sb = constp.tile([64, CT, 2, 128], F8, tag="wq")
                wk_sb = constp.tile([64, CT, 2, 128], F8, tag="wk")
                wv_sb = constp.tile([64, CT, 2, 128], F8, tag="wv")
                for w_dram, w_sb in ((wk, wk_sb), (wq, wq_sb), (wv, wv_sb)):
                    nc.gpsimd.dma_start(
                        out=w_sb[:],
                        in_=w_dram[:].rearrange(
                            "p (ct j m) -> p ct j m", ct=CT, j=2))
            else:
                wq_sb = constp.tile([128, C], F16, tag="wq")
                wk_sb = constp.tile([128, C], F16, tag="wk")
                wv_sb = constp.tile([128, C], F16, tag="wv")
                # k first (first consumer), on the fast sync HWDGE ring
                nc.sync.dma_start(out=wk_sb[:], in_=wk[:])
                nc.gpsimd.dma_start(out=wq_sb[:], in_=wq[:])
                nc.gpsimd.dma_start(out=wv_sb[:], in_=wv[:])
            wp_sb = constp.tile([128, C], F16, tag="wp")
            nc.gpsimd.dma_start(out=wp_sb[:], in_=wp[:])
            bq_sb = constp.tile([128, 1], F32, tag="bq")
            bk_sb = constp.tile([128, 1], F32, tag="bk")
            nc.gpsimd.dma_start(out=bq_sb[:], in_=bq[:])
            nc.gpsimd.dma_start(out=bk_sb[:], in_=bk[:])
            # 0/1 masks for the tail-only PE-based recip broadcast
            mlo = constp.tile([1, 128], F16, tag="mlo")
            mhi = constp.tile([1, 128], F16, tag="mhi")
            nc.vector.memset(mlo[:], 0.0)
            nc.vector.memset(mlo[0:1, 0:64], 1.0)
            nc.vector.memset(mhi[:], 0.0)
            nc.vector.memset(mhi[0:1, 64:128], 1.0)
            def emit_proj(pbi, yT_tile, pqt):
                # output projection for q-tile pqt (2 subtiles per DMA);
                # PSUM evacuation split between scalar and vector engines
                for sp in range(QT // 256):
                    osb = outp.tile([128, 2, C], F16, tag="osb")
                    for sj in range(2):
                        st = pqt * (QT // 128) + sp * 2 + sj
                        for n in range(C // QT):
                            pp = ps_mm.tile([128, QT], F32, tag="mm")
                            nc.tensor.matmul(
                                pp[:],
                                yT_tile[:, st * 128:(st + 1) * 128],
                                wp_sb[:, n * QT:(n + 1) * QT],
                                start=True, stop=True)
                            if n == 1 and sj == 1:
                                nc.scalar.activation(
                                    osb[:, sj, n * QT:(n + 1) * QT],
                                    pp[:], AF.Copy)
                            else:
                                nc.vector.tensor_copy(
                                    out=osb[:, sj, n * QT:(n + 1) * QT],
                                    in_=pp[:])
                    r0 = pbi * t + (pqt * (QT // 128) + sp * 2) * 128
                    nc.sync.dma_start(
                        out=out[r0:r0 + 256, :].rearrange(
                            "(s p) c -> p s c", p=128),
                        in_=osb[:])

            def emit_qkv(bi):
                # v_sb per 128-token subtile (VW layout, see above)
                v_sb = qkp.tile([128, nst * VW], F16, tag="v")
                v_view = v_sb[:].rearrange("p (s w) -> p s w", w=VW)
                nc.vector.memset(v_view[:, :, 64:66], 1.0)
                nc.vector.memset(v_view[:, :, 66:129], 0.0)
                qT_sb = qkp.tile([128, t], F16, tag="qT")
                kT_sb = qkp.tile([128, t], F16, tag="kT")

                xts = []
                for colt in range(t // QT):
                    if FP8_QKV:
                        xt = xin.tile([64, CT, 2, QT], F8, tag="xt")
                    else:
                        xt = xin.tile([128, CT, QT], F16, tag="xt")
                    csl = slice(bi * t + colt * QT, bi * t + (colt + 1) * QT)
                    if FP8_QKV:
                        if bi == 0 and colt == 0:
                            for ct in range(CT):
                                nc.sync.dma_start(
                                    out=xt[:, ct],
                                    in_=xT[ct * 64:(ct + 1) * 64, :, csl])
                        else:
                            nc.sync.dma_start(
                                out=xt[:],
                                in_=xT[:, :, csl].rearrange(
                                    "(ct p) j n -> p ct j n", p=64))
                    elif bi == 0 and colt == 0:
                        # split the very first load per contraction tile so
                        # the first matmul can start as soon as ct=0 lands
                        for ct in range(CT):
                            nc.sync.dma_start(
                                out=xt[:, ct, :],
                                in_=xT[ct * 128:(ct + 1) * 128, csl])
                    else:
                        nc.sync.dma_start(
                            out=xt[:],
                            in_=xT[:, csl].rearrange(
                                "(ct p) n -> p ct n", p=128))
                    xts.append(xt)

                def qkv_mm(ps, w_sb, xt, ct):
                    if FP8_QKV:
                        nc.tensor.matmul(
                            ps[:], w_sb[:, ct], xt[:, ct],
                            start=(ct == 0), stop=(ct == CT - 1),
                            perf_mode=mybir.MatmulPerfMode.DoubleRow)
                    else:
                        nc.tensor.matmul(
                            ps[:], w_sb[:, ct * 128:(ct + 1) * 128],
                            xt[:, ct, :],
                            start=(ct == 0), stop=(ct == CT - 1))

                for colt in range(t // QT):
                    xt = xts[colt]
                    csl = slice(colt * QT, (colt + 1) * QT)
                    for w_sb, dst, bias in (
                        (wk_sb, kT_sb, bk_sb), (wq_sb, qT_sb, bq_sb),
                    ):
                        ps = ps_mm.tile([128, QT], F32, tag="mm")
                        for ct in range(CT):
                            qkv_mm(ps, w_sb, xt, ct)
                        nc.vector.tensor_scalar_add(
                            out=dst[:, csl], in0=ps[:], scalar1=bias[:])
                    # V^T for this col tile, then transpose to natural layout
                    ps = ps_mm.tile([128, QT], F32, tag="mm")
                    for ct in range(CT):
                        qkv_mm(ps, wv_sb, xt, ct)
                    vt_col = smallp.tile([128, QT], F16, tag="vtcol")
                    nc.vector.tensor_copy(out=vt_col[:], in_=ps[:])
                    for sj in range(QT // 128):
                        st = colt * (QT // 128) + sj
                        vt_ps = ps_mm.tile([128, 128], F16, tag="mm")
                        nc.tensor.transpose(
                            vt_ps[:], vt_col[:, sj * 128:(sj + 1) * 128],
                            ident[:])
                        nc.vector.tensor_copy(
                            out=v_view[:, st, 0:64], in_=vt_ps[:, 0:64])
                        nc.vector.tensor_copy(
                            out=v_view[:, st, 129:193], in_=vt_ps[:, 64:128])
                return qT_sb, kT_sb, v_view

            def emit_attention(bi, qT_sb, kT_sb, v_view):
                nonlocal pending
                yT_sb = ytp.tile([128, t], F16, tag="yT")
                for qt in range(nq):
                    n_k = (qt + 1) * (QT // KT)   # k-tiles of 128
                    q0 = qt * QT
                    ya_a = ps_ya.tile([65, QT], F32, tag="ya", name="yaA")
                    ya_b = ps_ya.tile([128, QT], F32, tag="ya", name="yaB")
                    for kt in range(n_k):
                        lo = max(0, kt * KT - q0)
                        # S^T for both heads: head A on PE rows 0-63,
                        # head B on rows 64-127.
                        sg = ps_s.tile([128, 2 * QT], F32, tag="sg")
                        es = esp.tile([128, 2 * QT], F16, tag="es")
                        for h in range(2):
                            hsl = slice(h * 64, (h + 1) * 64)
                            nc.tensor.matmul(
                                sg[:, h * QT + lo:(h + 1) * QT],
                                kT_sb[hsl, kt * KT:(kt + 1) * KT],
                                qT_sb[hsl, q0 + lo:q0 + QT],
                                start=True, stop=True,
                            )
                        sg_v = sg[:].rearrange("p (h q) -> p h q", h=2)
                        es_v = es[:].rearrange("p (h q) -> p h q", h=2)
                        nc.scalar.activation(
                            es_v[:, :, lo:], sg_v[:, :, lo:],
                            AF.Exp,
                            scale=SCALE / (WS * WS) if FP8_QKV else SCALE)
                        if kt * KT >= q0:
                            # causal band select, both heads in one op
                            nc.gpsimd.affine_select(
                                out=es_v[:, :, lo:lo + KT],
                                in_=es_v[:, :, lo:lo + KT],
                                compare_op=mybir.AluOpType.is_ge,
                                fill=0.0,
                                base=0,
                                channel_multiplier=-1,
                                pattern=[[0, 2], [1, KT]],
                            )
                        nc.tensor.matmul(
                            ya_a[:, lo:QT],
                            v_view[:, kt, 0:65],
                            es[:, lo:QT],
                            start=(kt == 0), stop=(kt == n_k - 1),
                        )
                        nc.tensor.matmul(
                            ya_b[:, lo:QT],
                            v_view[:, kt, 65:193],
                            es[:, QT + lo:2 * QT],
                            start=(kt == 0), stop=(kt == n_k - 1),
                        )
                    if len(pending) >= 2:
                        emit_proj(*pending.pop(0))
                    # normalize: y = ya / sum.  Head A's sum sits at psum
                    # partition 64 (scalar engine hops it to partition 0);
                    # head B's sum is already at partition 0.
                    srow = smallp.tile([1, QT], F32, tag="srow")
                    nc.scalar.activation(
                        srow[0:1, :], ya_a[64:65, :], AF.Copy)
                    # head B's sum is at partition 0 -> plain vector copy
                    srow_b = smallp.tile([1, QT], F32, tag="srowb")
                    nc.vector.tensor_copy(
                        out=srow_b[0:1, :], in_=ya_b[0:1, :])
                    # [1/sumA | 1/sumB] in one row, one broadcast to all 128
                    # partitions (gpsimd only runs affine_select besides this,
                    # so its queue stays short)
                    rr = smallp.tile([1, 2 * QT], F32, tag="rr")
                    nc.vector.reciprocal_approx_fast(
                        out=rr[0:1, QT:2 * QT], in_=srow_b[0:1, :])
                    nc.vector.reciprocal_approx_fast(
                        out=rr[0:1, 0:QT], in_=srow[0:1, :])
                    if bi == b - 1 and qt == nq - 1:
                        # tail: broadcast via two masked K=1 matmuls on the
                        # PE (keeps it warm; gpsimd queue may be draining)
                        rr16 = smallp.tile([1, 2 * QT], F16, tag="rr16")
                        nc.vector.tensor_copy(out=rr16[:], in_=rr[:])
                        bc_ps = ps_mm.tile([128, QT], F32, tag="mm")
                        nc.tensor.matmul(
                            bc_ps[:], mlo[:], rr16[0:1, 0:QT],
                            start=True, stop=False)
                        nc.tensor.matmul(
                            bc_ps[:], mhi[:], rr16[0:1, QT:2 * QT],
                            start=False, stop=True)
                        bc = smallp.tile([128, QT], F32, tag="bc")
                        nc.vector.tensor_copy(out=bc[:], in_=bc_ps[:])
                        bca = bc[0:64, :]
                        bcb = bc[64:128, :]
                    else:
                        bc = smallp.tile([128, 2 * QT], F32, tag="bc")
                        nc.gpsimd.partition_broadcast(
                            bc[:], rr[0:1, :], channels=128)
                        bca = bc[0:64, 0:QT]
                        bcb = bc[64:128, QT:2 * QT]
                    nc.vector.tensor_mul(
                        out=yT_sb[0:64, q0:q0 + QT],
                        in0=ya_a[0:64, :], in1=bca)
                    nc.vector.tensor_mul(
                        out=yT_sb[64:128, q0:q0 + QT],
                        in0=ya_b[64:128, :], in1=bcb)
                    pending.append((bi, yT_sb, qt))

            # Software-pipeline one full batch ahead: qkv(bi+1) is emitted
            # before attention(bi), so the attention phase always has a deep
            # well of ready tensor work and never couples to fresh qkv.
            pending = []   # (bi, yT_tile, qt) whose proj is not yet emitted
            tiles = [None] * b
            tiles[0] = emit_qkv(0)
            for bi in range(b):
                if bi + 1 < b:
                    tiles[bi + 1] = emit_qkv(bi + 1)
                emit_attention(bi, *tiles[bi])
                tiles[bi] = None
            for p in pending:
                emit_proj(*p)

    nc.compile()
    return nc


NPF8 = mybir.dt.np(F8)


def _lhsT(w):
    """[C, 128] weight slice -> lhsT layout [128, C]."""
    return np.ascontiguousarray(
        w.reshape(CT, 128, 128).transpose(1, 0, 2).reshape(128, C)
    ).astype(np.float16)


def _lhsT8(w):
    """[C, 128] weight slice -> DoubleRow lhsT layout [64, 2C] fp8."""
    return np.ascontiguousarray(
        (w * WS).reshape(CT, 64, 2, 128).transpose(1, 0, 2, 3)
        .reshape(64, 2 * C)).astype(NPF8)


def _prep_inputs(x, W_attn, b_attn, W_proj, b_proj, b, t):
    bt = b * t
    if FP8_QKV:
        xT_full = np.ascontiguousarray(
            x.reshape(bt, CT, 64, 2).transpose(1, 2, 3, 0)
            .reshape(C // 2, 2, bt)).astype(NPF8)
        bscale = WS
        wp_full = (W_proj / WS).astype(np.float16)
        mk_w = _lhsT8
    else:
        xT_full = np.ascontiguousarray(
            x.reshape(bt, C).T).astype(np.float16)
        bscale = 1.0
        wp_full = W_proj.astype(np.float16)
        mk_w = _lhsT
    in_maps = []
    for c in range(NCORES):
        sl = slice(c * 128, (c + 1) * 128)
        in_maps.append({
            "xT": xT_full,
            "wq": mk_w(W_attn[:, sl]),
            "wk": mk_w(W_attn[:, 1024:2048][:, sl]),
            "wv": mk_w(W_attn[:, 2048:3072][:, sl]),
            "wp": np.ascontiguousarray(wp_full[sl, :]),
            "bq": np.ascontiguousarray(
                b_attn[sl].reshape(128, 1) * bscale).astype(np.float32),
            "bk": np.ascontiguousarray(
                b_attn[1024:2048][sl].reshape(128, 1) * bscale
            ).astype(np.float32),
        })
    return in_maps


def kernel(x, W_attn, b_attn, W_proj, b_proj, _trace=False):
    from concourse.bass_utils import run_bass_kernel_spmd

    x = np.asarray(x, dtype=np.float32)
    W_attn = np.asarray(W_attn, dtype=np.float32)
    b_attn = np.asarray(b_attn, dtype=np.float32)
    W_proj = np.asarray(W_proj, dtype=np.float32)
    b_proj = np.asarray(b_proj, dtype=np.float32)
    b, t, c = x.shape

    key = (b, t)
    if key not in _CACHED:
        _CACHED[key] = build_kernel(b, t)
    nc = _CACHED[key]

    in_maps = _prep_inputs(x, W_attn, b_attn, W_proj, b_proj, b, t)
    res = run_bass_kernel_spmd(
        nc, in_maps, core_ids=list(range(NCORES)), trace=_trace)

    acc = res.results[0]["out"].astype(np.float32)
    for r in res.results[1:]:
        acc = acc + r["out"].astype(np.float32)
    acc += b_attn[2048:3072] @ W_proj + b_proj
    out = acc.reshape(b, t, c)
    if _trace:
        kernel.last_result = res
    return out


# revision 57
# speedup vs baseline: 1.0187x; 1.0168x over previous
"""Causal self-attention Trainium2 kernel — tensor-parallel over heads on 8 NeuronCores.

Problem: B=4, T=2048, C=1024, H=16 heads (head_dim 64), fp32 in/out.
Sharding: 2 heads per core. Each core computes the qkv projection for its
128 head-dim columns, full causal attention for its 2 heads, and a partial
output projection (its 128 W_proj rows); fp16 partials are summed on host.

All matmul operands are fp16 (PE runs fp16 at 1 cycle/row, same as bf16,
with 10-bit mantissa); accumulation stays fp32 in PSUM, softmax
normalization in fp32. Engine assignment is explicit: scalar = exp + q/k
bias evacuation, vector = copies/normalize, gpsimd = causal mask +
partition broadcast + x loads, sync = output stores.
"""

import numpy as np

import concourse.bass as bass
import concourse.mybir as mybir
from concourse import bacc
from concourse.tile import TileContext
from concourse.masks import make_identity

# NOTE: walrus's --enable-ldw-opt rejects fp16 LDWEIGHTS ("InstLdweights is
# not compatible with LDW optimization"), so unlike the fp32r variant we run
# with the default ldw-opt=false.

F32 = mybir.dt.float32
F16 = mybir.dt.float16
F8 = mybir.dt.float8e4
AF = mybir.ActivationFunctionType

# fp8 DoubleRow QKV projection: x and W_attn are fed to the PE as fp8(e4m3)
# in DoubleRow pair-interleaved layout (2x matmul throughput).  W is
# prescaled by WS on the host to sit in fp8 range; the factor is folded out
# via the exp scale (q.k picks up WS^2) and W_proj (y picks up WS).
# Measured on the real inputs: fp8 q/k loses ~4.5e-2 rel err (the exp
# amplifies operand rounding), so this stays off.
FP8_QKV = False
WS = 32.0

B, T, C, H = 4, 2048, 1024, 16
HD = 64
NCORES = 8
CT = C // 128          # 8 contraction tiles
QT = 512               # q tile (free dim of S^T matmuls)
KT = 128               # k tile (partition dim of S^T)
SCALE = 1.0 / np.sqrt(HD)
# v subtile layout: [vA(64) | onesA(1) | onesB(1) | junk(63) | vB(64)].
# Head A's PV stationary is cols 0:65 ([vA|onesA] -> ya_A dims at psum
# partitions 0-63, sum at 64). Head B's is cols 65:193
# ([onesB|junk63|vB] -> sum at partition 0, dims at partitions 64-127), so
# the normalized y of head B can be written to yT[64:128] without any
# cross-partition move.
VW = 193

_CACHED = {}


def build_kernel(b=B, t=T):
    """Build the per-core SPMD program. t must be a multiple of 512."""
    from concourse import hw_specs

    # Calibrate the tile scheduler's cost model to measured per-op rates
    # (NTFF traces show ACT ~13% and Pool ~2x above the stock model); this
    # only changes the static instruction interleave, not semantics.
    # Restored after compile so nothing leaks.
    _ET = mybir.EngineType
    _saved = dict(hw_specs.TRN2Spec.CYCLE_T)
    hw_specs.TRN2Spec.CYCLE_T[_ET.Activation] = 1e9 / 1.05e9
    hw_specs.TRN2Spec.CYCLE_T[_ET.Pool] = 1e9 / 0.6e9
    try:
        return _build_kernel_inner(b, t)
    finally:
        hw_specs.TRN2Spec.CYCLE_T.clear()
        hw_specs.TRN2Spec.CYCLE_T.update(_saved)


def _build_kernel_inner(b, t):
    assert t % QT == 0
    nq = t // QT           # q-tiles per sequence
    nst = t // 128         # 128-token subtiles per sequence
    bt = b * t

    nc = bacc.Bacc("TRN2", target_bir_lowering=False, debug=False,
                   num_devices=NCORES)

    if FP8_QKV:
        # x in DoubleRow layout: [ct*64+p, j, tok] = x[tok, ct*128+2p+j]
        xT = nc.dram_tensor("xT", [C // 2, 2, bt], F8, kind="ExternalInput")
        # W in DoubleRow lhsT layout: [p, ct, j, m] = WS*W[ct*128+2p+j, m]
        wq = nc.dram_tensor("wq", [64, 2 * C], F8, kind="ExternalInput")
        wk = nc.dram_tensor("wk", [64, 2 * C], F8, kind="ExternalInput")
        wv = nc.dram_tensor("wv", [64, 2 * C], F8, kind="ExternalInput")
    else:
        xT = nc.dram_tensor("xT", [C, bt], F16, kind="ExternalInput")
        # weights in lhsT layout ([p, ct*128+m] = W[ct*128+p, m]),
        # pre-arranged on host so the load is one contiguous DMA
        wq = nc.dram_tensor("wq", [128, C], F16, kind="ExternalInput")
        wk = nc.dram_tensor("wk", [128, C], F16, kind="ExternalInput")
        wv = nc.dram_tensor("wv", [128, C], F16, kind="ExternalInput")
    wp = nc.dram_tensor("wp", [128, C], F16, kind="ExternalInput")
    bq = nc.dram_tensor("bq", [128, 1], F32, kind="ExternalInput")
    bk = nc.dram_tensor("bk", [128, 1], F32, kind="ExternalInput")
    out = nc.dram_tensor("out", [bt, C], F16, kind="ExternalOutput")

    with TileContext(nc) as tc:
        with (
            tc.tile_pool(name="const", bufs=1) as constp,
            tc.tile_pool(name="xin", bufs=4) as xin,
            tc.tile_pool(name="qk", bufs=3) as qkp,
            tc.tile_pool(name="es", bufs=4) as esp,
            tc.tile_pool(name="yt", bufs=2) as ytp,
            tc.tile_pool(name="small", bufs=3) as smallp,
            tc.tile_pool(name="outsb", bufs=3) as outp,
            tc.tile_pool(name="ps_s", bufs=2, space="PSUM") as ps_s,
            tc.tile_pool(name="ps_ya", bufs=2, space="PSUM") as ps_ya,
            tc.tile_pool(name="ps_mm", bufs=2, space="PSUM") as ps_mm,
        ):
            # ---- constants / weights ----
            ident = constp.tile([128, 128], F16, tag="ident")
            make_identity(nc, ident[:])
            if FP8_QKV:
                wq_sb = constp.tile([64, CT, 2, 128], F8, tag="wq")
                wk_sb = constp.tile([64, CT, 2, 128], F8, tag="wk")
                wv_sb = constp.tile([64, CT, 2, 128], F8, tag="wv")
                for w_dram, w_sb in ((wk, wk_sb), (wq, wq_sb), (wv, wv_sb)):
                    nc.gpsimd.dma_start(
                        out=w_sb[:],
                        in_=w_dram[:].rearrange(
                            "p (ct j m) -> p ct j m", ct=CT, j=2))
            else:
                wq_sb = constp.tile([128, C], F16, tag="wq")
                wk_sb = constp.tile([128, C], F16, tag="wk")
                wv_sb = constp.tile([128, C], F16, tag="wv")
                # k first (first consumer), on the fast sync HWDGE ring
                nc.sync.dma_start(out=wk_sb[:], in_=wk[:])
                nc.gpsimd.dma_start(out=wq_sb[:], in_=wq[:])
                nc.gpsimd.dma_start(out=wv_sb[:], in_=wv[:])
            wp_sb = constp.tile([128, C], F16, tag="wp")
            nc.gpsimd.dma_start(out=wp_sb[:], in_=wp[:])
            bq_sb = constp.tile([128, 1], F32, tag="bq")
            bk_sb = constp.tile([128, 1], F32, tag="bk")
            nc.gpsimd.dma_start(out=bq_sb[:], in_=bq[:])
            nc.gpsimd.dma_start(out=bk_sb[:], in_=bk[:])
            def emit_proj(pbi, yT_tile, pqt):
                # output projection for q-tile pqt (2 subtiles per DMA);
                # PSUM evacuation split between scalar and vector engines
                for sp in range(QT // 256):
                    osb = outp.tile([128, 2, C], F16, tag="osb")
                    for sj in range(2):
                        st = pqt * (QT // 128) + sp * 2 + sj
                        for n in range(C // QT):
                            pp = ps_mm.tile([128, QT], F32, tag="mm")
                            nc.tensor.matmul(
                                pp[:],
                                yT_tile[:, st * 128:(st + 1) * 128],
                                wp_sb[:, n * QT:(n + 1) * QT],
                                start=True, stop=True)
                            if n == 1 and sj == 1:
                                nc.scalar.activation(
                                    osb[:, sj, n * QT:(n + 1) * QT],
                                    pp[:], AF.Copy)
                            else:
                                nc.vector.tensor_copy(
                                    out=osb[:, sj, n * QT:(n + 1) * QT],
                                    in_=pp[:])
                    r0 = pbi * t + (pqt * (QT // 128) + sp * 2) * 128
                    nc.sync.dma_start(
                        out=out[r0:r0 + 256, :].rearrange(
                            "(s p) c -> p s c", p=128),
                        in_=osb[:])

            def emit_qkv(bi):
                # v_sb per 128-token subtile (VW layout, see above)
                v_sb = qkp.tile([128, nst * VW], F16, tag="v")
                v_view = v_sb[:].rearrange("p (s w) -> p s w", w=VW)
                nc.vector.memset(v_view[:, :, 64:66], 1.0)
                nc.vector.memset(v_view[:, :, 66:129], 0.0)
                qT_sb = qkp.tile([128, t], F16, tag="qT")
                kT_sb = qkp.tile([128, t], F16, tag="kT")

                xts = []
                for colt in range(t // QT):
                    if FP8_QKV:
                        xt = xin.tile([64, CT, 2, QT], F8, tag="xt")
                    else:
                        xt = xin.tile([128, CT, QT], F16, tag="xt")
                    csl = slice(bi * t + colt * QT, bi * t + (colt + 1) * QT)
                    if FP8_QKV:
                        if bi == 0 and colt == 0:
                            for ct in range(CT):
                                nc.sync.dma_start(
                                    out=xt[:, ct],
                                    in_=xT[ct * 64:(ct + 1) * 64, :, csl])
                        else:
                            nc.sync.dma_start(
                                out=xt[:],
                                in_=xT[:, :, csl].rearrange(
                                    "(ct p) j n -> p ct j n", p=64))
                    elif bi == 0 and colt == 0:
                        # split the very first load per contraction tile so
                        # the first matmul can start as soon as ct=0 lands
                        for ct in range(CT):
                            nc.sync.dma_start(
                                out=xt[:, ct, :],
                                in_=xT[ct * 128:(ct + 1) * 128, csl])
                    else:
                        nc.sync.dma_start(
                            out=xt[:],
                            in_=xT[:, csl].rearrange(
                                "(ct p) n -> p ct n", p=128))
                    xts.append(xt)

                def qkv_mm(ps, w_sb, xt, ct):
                    if FP8_QKV:
                        nc.tensor.matmul(
                            ps[:], w_sb[:, ct], xt[:, ct],
                            start=(ct == 0), stop=(ct == CT - 1),
                            perf_mode=mybir.MatmulPerfMode.DoubleRow)
                    else:
                        nc.tensor.matmul(
                            ps[:], w_sb[:, ct * 128:(ct + 1) * 128],
                            xt[:, ct, :],
                            start=(ct == 0), stop=(ct == CT - 1))

                for colt in range(t // QT):
                    xt = xts[colt]
                    csl = slice(colt * QT, (colt + 1) * QT)
                    for w_sb, dst, bias in (
                        (wk_sb, kT_sb, bk_sb), (wq_sb, qT_sb, bq_sb),
                    ):
                        ps = ps_mm.tile([128, QT], F32, tag="mm")
                        for ct in range(CT):
                            qkv_mm(ps, w_sb, xt, ct)
                        nc.vector.tensor_scalar_add(
                            out=dst[:, csl], in0=ps[:], scalar1=bias[:])
                    # V^T for this col tile, then transpose to natural layout
                    ps = ps_mm.tile([128, QT], F32, tag="mm")
                    for ct in range(CT):
                        qkv_mm(ps, wv_sb, xt, ct)
                    vt_col = smallp.tile([128, QT], F16, tag="vtcol")
                    nc.vector.tensor_copy(out=vt_col[:], in_=ps[:])
                    for sj in range(QT // 128):
                        st = colt * (QT // 128) + sj
                        vt_ps = ps_mm.tile([128, 128], F16, tag="mm")
                        nc.tensor.transpose(
                            vt_ps[:], vt_col[:, sj * 128:(sj + 1) * 128],
                            ident[:])
                        nc.vector.tensor_copy(
                            out=v_view[:, st, 0:64], in_=vt_ps[:, 0:64])
                        nc.vector.tensor_copy(
                            out=v_view[:, st, 129:193], in_=vt_ps[:, 64:128])
                return qT_sb, kT_sb, v_view

            def emit_attention(bi, qT_sb, kT_sb, v_view):
                nonlocal pending
                yT_sb = ytp.tile([128, t], F16, tag="yT")
                for qt in range(nq):
                    n_k = (qt + 1) * (QT // KT)   # k-tiles of 128
                    q0 = qt * QT
                    ya_a = ps_ya.tile([65, QT], F32, tag="ya", name="yaA")
                    ya_b = ps_ya.tile([128, QT], F32, tag="ya", name="yaB")
                    for kt in range(n_k):
                        lo = max(0, kt * KT - q0)
                        # S^T for both heads: head A on PE rows 0-63,
                        # head B on rows 64-127.
                        sg = ps_s.tile([128, 2 * QT], F32, tag="sg")
                        es = esp.tile([128, 2 * QT], F16, tag="es")
                        for h in range(2):
                            hsl = slice(h * 64, (h + 1) * 64)
                            nc.tensor.matmul(
                                sg[:, h * QT + lo:(h + 1) * QT],
                                kT_sb[hsl, kt * KT:(kt + 1) * KT],
                                qT_sb[hsl, q0 + lo:q0 + QT],
                                start=True, stop=True,
                            )
                        sg_v = sg[:].rearrange("p (h q) -> p h q", h=2)
                        es_v = es[:].rearrange("p (h q) -> p h q", h=2)
                        nc.scalar.activation(
                            es_v[:, :, lo:], sg_v[:, :, lo:],
                            AF.Exp,
                            scale=SCALE / (WS * WS) if FP8_QKV else SCALE)
                        if kt * KT >= q0:
                            # causal band select, both heads in one op
                            nc.gpsimd.affine_select(
                                out=es_v[:, :, lo:lo + KT],
                                in_=es_v[:, :, lo:lo + KT],
                                compare_op=mybir.AluOpType.is_ge,
                                fill=0.0,
                                base=0,
                                channel_multiplier=-1,
                                pattern=[[0, 2], [1, KT]],
                            )
                        nc.tensor.matmul(
                            ya_a[:, lo:QT],
                            v_view[:, kt, 0:65],
                            es[:, lo:QT],
                            start=(kt == 0), stop=(kt == n_k - 1),
                        )
                        nc.tensor.matmul(
                            ya_b[:, lo:QT],
                            v_view[:, kt, 65:193],
                            es[:, QT + lo:2 * QT],
                            start=(kt == 0), stop=(kt == n_k - 1),
                        )
                    if len(pending) >= 2:
                        emit_proj(*pending.pop(0))
                    # normalize: y = ya / sum.  Head A's sum sits at psum
                    # partition 64 (scalar engine hops it to partition 0);
                    # head B's sum is already at partition 0.
                    srow = smallp.tile([1, QT], F32, tag="srow")
                    nc.scalar.activation(
                        srow[0:1, :], ya_a[64:65, :], AF.Copy)
                    # head B's sum is at partition 0 -> plain vector copy
                    srow_b = smallp.tile([1, QT], F32, tag="srowb")
                    nc.vector.tensor_copy(
                        out=srow_b[0:1, :], in_=ya_b[0:1, :])
                    # [1/sumA | 1/sumB] in one row, one broadcast to all 128
                    # partitions (gpsimd only runs affine_select besides this,
                    # so its queue stays short)
                    rr = smallp.tile([1, 2 * QT], F32, tag="rr")
                    nc.vector.reciprocal_approx_fast(
                        out=rr[0:1, QT:2 * QT], in_=srow_b[0:1, :])
                    nc.vector.reciprocal_approx_fast(
                        out=rr[0:1, 0:QT], in_=srow[0:1, :])
                    bc = smallp.tile([128, 2 * QT], F32, tag="bc")
                    nc.gpsimd.partition_broadcast(
                        bc[:], rr[0:1, :], channels=128)
                    nc.vector.tensor_mul(
                        out=yT_sb[0:64, q0:q0 + QT],
                        in0=ya_a[0:64, :], in1=bc[0:64, 0:QT])
                    nc.vector.tensor_mul(
                        out=yT_sb[64:128, q0:q0 + QT],
                        in0=ya_b[64:128, :], in1=bc[64:128, QT:2 * QT])
                    pending.append((bi, yT_sb, qt))

            # Software-pipeline one full batch ahead: qkv(bi+1) is emitted
            # before attention(bi), so the attention phase always has a deep
            # well of ready tensor work and never couples to fresh qkv.
            pending = []   # (bi, yT_tile, qt) whose proj is not yet emitted
            tiles = [None] * b
            tiles[0] = emit_qkv(0)
            for bi in range(b):
                if bi + 1 < b:
                    tiles[bi + 1] = emit_qkv(bi + 1)
                emit_attention(bi, *tiles[bi])
                tiles[bi] = None
            for p in pending:
                emit_proj(*p)

    nc.compile()
    return nc


NPF8 = mybir.dt.np(F8)


def _lhsT(w):
    """[C, 128] weight slice -> lhsT layout [128, C]."""
    return np.ascontiguousarray(
        w.reshape(CT, 128, 128).transpose(1, 0, 2).reshape(128, C)
    ).astype(np.float16)


def _lhsT8(w):
    """[C, 128] weight slice -> DoubleRow lhsT layout [64, 2C] fp8."""
    return np.ascontiguousarray(
        (w * WS).reshape(CT, 64, 2, 128).transpose(1, 0, 2, 3)
        .reshape(64, 2 * C)).astype(NPF8)


def _prep_inputs(x, W_attn, b_attn, W_proj, b_proj, b, t):
    bt = b * t
    if FP8_QKV:
        xT_full = np.ascontiguousarray(
            x.reshape(bt, CT, 64, 2).transpose(1, 2, 3, 0)
            .reshape(C // 2, 2, bt)).astype(NPF8)
        bscale = WS
        wp_full = (W_proj / WS).astype(np.float16)
        mk_w = _lhsT8
    else:
        xT_full = np.ascontiguousarray(
            x.reshape(bt, C).T).astype(np.float16)
        bscale = 1.0
        wp_full = W_proj.astype(np.float16)
        mk_w = _lhsT
    in_maps = []
    for c in range(NCORES):
        sl = slice(c * 128, (c + 1) * 128)
        in_maps.append({
            "xT": xT_full,
            "wq": mk_w(W_attn[:, sl]),
            "wk": mk_w(W_attn[:, 1024:2048][:, sl]),
            "wv": mk_w(W_attn[:, 2048:3072][:, sl]),
            "wp": np.ascontiguousarray(wp_full[sl, :]),
            "bq": np.ascontiguousarray(
                b_attn[sl].reshape(128, 1) * bscale).astype(np.float32),
            "bk": np.ascontiguousarray(
                b_attn[1024:2048][sl].reshape(128, 1) * bscale
            ).astype(np.float32),
        })
    return in_maps


def kernel(x, W_attn, b_attn, W_proj, b_proj, _trace=False):
    from concourse.bass_utils import run_bass_kernel_spmd

    x = np.asarray(x, dtype=np.float32)
    W_attn = np.asarray(W_attn, dtype=np.float32)
    b_attn = np.asarray(b_attn, dtype=np.float32)
    W_proj = np.asarray(W_proj, dtype=np.float32)
    b_proj = np.asarray(b_proj, dtype=np.float32)
    b, t, c = x.shape

    key = (b, t)
    if key not in _CACHED:
        _CACHED[key] = build_kernel(b, t)
    nc = _CACHED[key]

    in_maps = _prep_inputs(x, W_attn, b_attn, W_proj, b_proj, b, t)
    res = run_bass_kernel_spmd(
        nc, in_maps, core_ids=list(range(NCORES)), trace=_trace)

    acc = res.results[0]["out"].astype(np.float32)
    for r in res.results[1:]:
        acc = acc + r["out"].astype(np.float32)
    acc += b_attn[2048:3072] @ W_proj + b_proj
    out = acc.reshape(b, t, c)
    if _trace:
        kernel.last_result = res
    return out
